# revision 29
# baseline (speedup 1.0000x reference)
"""Chebyshev graph convolution (K=3) on 8 Trainium2 NeuronCores.

Strategy (1D destination partitioning):
- Nodes (destination rows) sharded across 8 cores: core c owns rows
  [c*6250, (c+1)*6250).  Edges partitioned by destination so segment_sum is
  local; per SpMM step the updated node features are AllGather'ed so each
  core can gather arbitrary source rows.
- SpMM on-chip: per 128-destination "pair" (2 halves of 64), edge messages
  are fetched with dma_gather (one gathered 512B row per edge, batches of
  128 edges land one-per-partition), a one-hot selection matrix
  sel[e, d] = vals[e] * (dloc[e] == d) is built on DVE, and the TensorEngine
  computes psum[d, :] += sel.T @ gathered — i.e. the scatter-add.
- Node features live in a partition-major padded layout: node
  (core c, pair j, lane p) -> table row (c*128+p)*49 + j, 128 f32 per row
  (512B, dma_gather elem granularity).  This makes the per-step shard
  writeback a single contiguous [128, 6272] DMA instead of a 6272-descriptor
  strided write.
- dma_gather uses int16 indices (<= 1024 per call), so the 50176-row table
  is addressed in two halves; edges are grouped by (dest pair, dest
  half-of-pair, source half) and padded to 128-edge batches with val=0
  edges.  Batch counts are maxed across cores so all 8 cores run one SPMD
  program.
"""

import sys

if "/opt/trn_rl_repo" not in sys.path:
    sys.path.insert(0, "/opt/trn_rl_repo")

import numpy as np

N_NODES = 50000
D = 96
C = 8  # cores
SH = N_NODES // C  # 6250 rows per core
PAIRS = 49  # ceil(6250/128)
PADSH = PAIRS * 128  # 6272
NPAD = C * PADSH  # 50176 padded table rows
HALF = NPAD // 2  # 25088
PADC = 128  # padded feature columns (512B rows for dma_gather)
CH = 2  # pairs per chunk (gather/sel granularity)

last_results = None  # BassKernelResults of the most recent run (for profiling)


def _row_of_node(g):
    """node id -> padded table row: (c*128 + p)*49 + j for g = c*6250 + j*128 + p."""
    c, r = g // SH, g % SH
    j, p = r // 128, r % 128
    return (c * 128 + p) * PAIRS + j


def _plan_chunks(NB):
    """Global batch layout.  NB[j, q] = batches for (pair j, src half q).
    Batch order: chunk-major, then q, then pair."""
    chunks = [list(range(i, min(i + CH, PAIRS))) for i in range(0, PAIRS, CH)]
    plan = []
    B = 0
    for pj in chunks:
        qspans = {}
        groups = []
        for q in (0, 1):
            q0 = B
            for j in pj:
                nb = int(NB[j, q])
                groups.append((q, j, B, nb))
                B += nb
            qspans[q] = (q0, B)
        plan.append(dict(pairs=pj, groups=groups, qspans=qspans, b0=qspans[0][0], b1=B))
    return chunks, plan, B


def _preprocess(rows, cols, vals):
    """Sort/partition edges, build per-core padded batch arrays."""
    rows = np.asarray(rows).astype(np.int64)
    cols = np.asarray(cols).astype(np.int64)
    vals = np.asarray(vals).astype(np.float32)

    order = np.argsort(rows, kind="stable")
    r_s, c_s, v_s = rows[order], cols[order], vals[order]
    core_bounds = np.searchsorted(r_s, np.arange(C + 1) * SH)

    per_core = []
    counts = np.zeros((C, PAIRS, 2), np.int64)
    for c in range(C):
        s, e = core_bounds[c], core_bounds[c + 1]
        ld = (r_s[s:e] - c * SH).astype(np.int64)
        j = ld // 128
        d128 = (ld % 128).astype(np.float32)
        prow = _row_of_node(c_s[s:e])
        q = (prow >= HALF).astype(np.int64)
        lidx = (prow - q * HALF).astype(np.int64)
        np.add.at(counts[c], (j, q), 1)
        per_core.append((j, q, d128, lidx, v_s[s:e]))

    NB = -(-counts.max(axis=0) // 128)  # ceil
    NB[:, 0] = np.maximum(NB[:, 0], 1)  # every pair has >=1 batch
    chunks, plan, TOTB = _plan_chunks(NB)

    # batch offset of each (j, q) group
    B0 = np.zeros((PAIRS, 2), np.int64)
    for ch in plan:
        for (q, j, b0, nb) in ch["groups"]:
            B0[j, q] = b0

    core_arrays = []
    for c in range(C):
        j, q, d128, lidx, v = per_core[c]
        g_b0 = B0[j, q]  # per-edge group batch offset
        o = np.argsort(g_b0, kind="stable")
        g_sorted = g_b0[o]
        uniq, starts, cnts = np.unique(g_sorted, return_index=True, return_counts=True)
        pos = np.arange(g_sorted.size) - np.repeat(starts, cnts)
        slot = g_sorted * 128 + pos  # global edge slot

        lidx_flat = np.zeros(TOTB * 128, np.int16)
        dloc_col = np.zeros((128, TOTB), np.float32)
        vals_col = np.zeros((128, TOTB), np.float32)
        lane = slot % 128
        bb = slot // 128
        lidx_flat[slot] = lidx[o].astype(np.int16)
        dloc_col[lane, bb] = d128[o]
        vals_col[lane, bb] = v[o]

        # wrapped int16 index tensor: per (chunk, q) span, idx i -> [i%16, i//16]
        widx = np.zeros((16, TOTB * 8), np.int16)
        for ch in plan:
            for qq in (0, 1):
                b0, b1 = ch["qspans"][qq]
                if b1 == b0:
                    continue
                seg = lidx_flat[b0 * 128:b1 * 128]
                n = seg.size
                widx[np.arange(n) % 16, b0 * 8 + np.arange(n) // 16] = seg
        widx = np.tile(widx, (8, 1))
        core_arrays.append((widx, dloc_col, vals_col))

    return chunks, plan, TOTB, core_arrays


def _build_program(plan, TOTB):
    import os
    from concourse import bass, bacc, mybir
    import concourse.tile as tile

    no_cc = bool(int(os.environ.get("CHEB_NO_CC", "0")))
    n_steps = int(os.environ.get("CHEB_STEPS", "3"))
    no_final = bool(int(os.environ.get("CHEB_NO_FINAL", "0")))
    maxch = int(os.environ.get("CHEB_MAXCH", "9999"))
    nqueues = int(os.environ.get("CHEB_QUEUES", "4"))

    f32 = mybir.dt.float32
    nc = bacc.Bacc("TRN2", target_bir_lowering=False, num_devices=C,
                   num_swdge_queues=nqueues)
    gq = [0]  # round-robin gather queue counter

    tbl0 = nc.dram_tensor("tbl0", [NPAD, PADC], f32, kind="ExternalInput")
    hsh_d = nc.dram_tensor("hsh", [128, PAIRS * PADC], f32, kind="ExternalInput")
    widx_d = nc.dram_tensor("widx", [128, TOTB * 8], mybir.dt.int16, kind="ExternalInput")
    dloc_d = nc.dram_tensor("dloc", [128, TOTB], f32, kind="ExternalInput")
    wval_d = nc.dram_tensor("wval", [128, TOTB], f32, kind="ExternalInput")
    iota_d = nc.dram_tensor("iota128", [128, 128], f32, kind="ExternalInput")
    ident_d = nc.dram_tensor("ident", [128, 128], f32, kind="ExternalInput")
    wmat_d = nc.dram_tensor("wmat", [D, D], f32, kind="ExternalInput")
    bias_d = nc.dram_tensor("biasb", [128, D], f32, kind="ExternalInput")
    out_d = nc.dram_tensor("out", [SH, D], f32, kind="ExternalOutput")

    tsh = [nc.dram_tensor(f"tsh{k}", [128, PAIRS * PADC], f32, kind="Internal")
           for k in (1, 2)]
    tfull = [nc.dram_tensor(f"tfull{k}", [NPAD, PADC], f32, kind="Internal",
                            addr_space="Shared") for k in (1, 2)]
    rg = [list(range(C))]

    with tile.TileContext(nc) as tc:
        with (
            tc.tile_pool(name="persist", bufs=1) as pp,
            tc.tile_pool(name="xgp", bufs=2) as xgp,
            tc.tile_pool(name="selp", bufs=2) as selp,
            tc.tile_pool(name="psum", bufs=4, space="PSUM") as psp,
            tc.tile_pool(name="psum2", bufs=2, space="PSUM") as psp2,
        ):
            widx_t = pp.tile([128, TOTB * 8], mybir.dt.int16)
            nc.sync.dma_start(out=widx_t[:], in_=widx_d[:, :])
            dloc_t = pp.tile([128, TOTB], f32)
            nc.sync.dma_start(out=dloc_t[:], in_=dloc_d[:, :])
            wval_t = pp.tile([128, TOTB], f32)
            nc.sync.dma_start(out=wval_t[:], in_=wval_d[:, :])
            iota_t = pp.tile([128, 128], f32)
            nc.sync.dma_start(out=iota_t[:], in_=iota_d[:, :])
            ident_t = pp.tile([128, 128], f32)
            nc.sync.dma_start(out=ident_t[:], in_=ident_d[:, :])
            wmat_t = pp.tile([D, D], f32)
            nc.sync.dma_start(out=wmat_t[:], in_=wmat_d[:, :])
            bias_t = pp.tile([128, D], f32)
            nc.sync.dma_start(out=bias_t[:], in_=bias_d[:, :])

            # T tiles are padded [128, 49*128] so the shard writeback is one
            # contiguous DMA; compute views touch only cols 0:96 of each pair.
            Tp = pp.tile([128, PAIRS * PADC], f32, tag="Tp")
            Tc = pp.tile([128, PAIRS * PADC], f32, tag="Tc")
            U = pp.tile([128, PAIRS * D], f32, tag="U")
            S = pp.tile([128, PAIRS * D], f32, tag="S")

            def v3(t):  # [128, PAIRS, 96] view of a padded tile
                return t[:].rearrange("p (j f) -> p j f", j=PAIRS)[:, :, 0:D]

            nc.gpsimd.memset(Tc[:], 0.0)  # pad cols stay 0 forever
            nc.sync.dma_start(out=Tp[:], in_=hsh_d[:, :])  # T0 = H (padded layout)
            TpV, TcV = v3(Tp), v3(Tc)
            U3 = U[:].rearrange("p (j f) -> p j f", j=PAIRS)
            S3 = S[:].rearrange("p (j f) -> p j f", j=PAIRS)
            nc.vector.tensor_copy(out=U3, in_=TpV)

            def spmm(table):
                """S <- spmm over this core's edges, gathering rows of `table`."""
                for ch in plan[:maxch]:
                    b0c, b1c = ch["b0"], ch["b1"]
                    nbc = b1c - b0c
                    xg = xgp.tile([128, nbc * 128], f32, tag="xg")
                    xg3 = xg[:].rearrange("p (b f) -> p b f", b=nbc)
                    for q in (0, 1):
                        s0, s1 = ch["qspans"][q]
                        # dma_gather accepts at most 1024 indices per call
                        for g0 in range(s0, s1, 8):
                            g1 = min(g0 + 8, s1)
                            nc.gpsimd.dma_gather(
                                out_ap=xg3[:, g0 - b0c:g1 - b0c, :],
                                in_ap=table[q * HALF:(q + 1) * HALF, :],
                                idxs_ap=widx_t[:, g0 * 8:g1 * 8],
                                num_idxs=(g1 - g0) * 128,
                                num_idxs_reg=(g1 - g0) * 128,
                                elem_size=PADC,
                                queue_num=gq[0] % nqueues,
                            )
                            gq[0] += 1
                    # sel[e, d] = vals[e] * (dloc[e] == d), one fused DVE
                    # tensor_scalar per batch (2x mode, pipelines with matmuls)
                    sel = selp.tile([128, nbc * 128], f32, tag="sel")
                    for b in range(nbc):
                        nc.vector.tensor_scalar(
                            out=sel[:, b * 128:(b + 1) * 128],
                            in0=iota_t[:],
                            scalar1=dloc_t[:, b0c + b:b0c + b + 1],
                            scalar2=wval_t[:, b0c + b:b0c + b + 1],
                            op0=mybir.AluOpType.is_equal,
                            op1=mybir.AluOpType.mult,
                        )
                    for j in ch["pairs"]:
                        ps = psp.tile([128, D], f32, tag="ps")
                        bl = []
                        for (q, jj, gb0, gnb) in ch["groups"]:
                            if jj == j:
                                bl.extend(range(gb0 - b0c, gb0 - b0c + gnb))
                        for i, b in enumerate(bl):
                            nc.tensor.matmul(
                                out=ps[:, :],
                                lhsT=sel[:, b * 128:(b + 1) * 128],
                                rhs=xg[:, b * 128:b * 128 + D],
                                start=(i == 0),
                                stop=(i == len(bl) - 1),
                            )
                        nc.scalar.copy(out=S[:, j * D:(j + 1) * D], in_=ps[:])

            def writeback(k):
                """Tc -> tsh[k] (one contiguous DMA) -> AllGather -> tfull[k]."""
                nc.sync.dma_start(out=tsh[k][:, :], in_=Tc[:])
                nc.gpsimd.collective_compute(
                    "AllGather",
                    mybir.AluOpType.bypass,
                    ins=[tsh[k][:, :]],
                    outs=[tfull[k][:, :]],
                    replica_groups=rg,
                )

            MUL, SUB, ADD = (mybir.AluOpType.mult, mybir.AluOpType.subtract,
                             mybir.AluOpType.add)

            # ---- k=1 : T1 = 2*spmm(H) - T0
            spmm(tbl0)
            nc.vector.scalar_tensor_tensor(
                out=TcV, in0=S3, scalar=2.0, in1=TpV, op0=MUL, op1=SUB)
            nc.vector.tensor_tensor(out=U3, in0=U3, in1=TcV, op=ADD)

            if n_steps >= 2:
                # ---- k=2 : T2 = 2*(2*spmm(T1) - T1) - T0
                if not no_cc:
                    writeback(0)
                spmm(tbl0 if no_cc else tfull[0])
                nc.vector.scalar_tensor_tensor(
                    out=S3, in0=S3, scalar=2.0, in1=TcV, op0=MUL, op1=SUB)
                nc.vector.scalar_tensor_tensor(
                    out=TpV, in0=S3, scalar=2.0, in1=TpV, op0=MUL, op1=SUB)
                Tp, Tc = Tc, Tp
                TpV, TcV = TcV, TpV
                nc.vector.tensor_tensor(out=U3, in0=U3, in1=TcV, op=ADD)

            if n_steps >= 3:
                # ---- k=3 : T3 = 2*(2*spmm(T2) - T2) - T1
                if not no_cc:
                    writeback(1)
                spmm(tbl0 if no_cc else tfull[1])
                nc.vector.scalar_tensor_tensor(
                    out=S3, in0=S3, scalar=2.0, in1=TcV, op0=MUL, op1=SUB)
                nc.vector.scalar_tensor_tensor(
                    out=TpV, in0=S3, scalar=2.0, in1=TpV, op0=MUL, op1=SUB)
                nc.vector.tensor_tensor(out=U3, in0=U3, in1=TpV, op=ADD)

            # ---- out = U @ W + bias, written back per pair
            O = S  # S is dead, reuse as output staging
            for j in range(PAIRS) if not no_final else []:
                pt = psp2.tile([128, 128], f32, tag="pt")
                nc.tensor.transpose(
                    out=pt[0:D, :], in_=U[:, j * D:(j + 1) * D], identity=ident_t[:])
                ut = selp.tile([128, 128], f32, tag="ut")
                nc.scalar.copy(out=ut[0:D, :], in_=pt[0:D, :])
                po = psp2.tile([128, D], f32, tag="po")
                nc.tensor.matmul(
                    out=po[:], lhsT=ut[0:D, :], rhs=wmat_t[:, :],
                    start=True, stop=True)
                nc.vector.tensor_tensor(
                    out=O[:, j * D:(j + 1) * D], in0=po[:], in1=bias_t[:], op=ADD)
                r1 = min((j + 1) * 128, SH)
                eng = nc.sync if j % 2 == 0 else nc.scalar
                eng.dma_start(
                    out=out_d[j * 128:r1, :],
                    in_=O[0:r1 - j * 128, j * D:(j + 1) * D],
                )

    nc.compile()
    return nc


def kernel(rows, cols, vals, H, W, bias):
    global last_results
    import os
    from concourse.bass_utils import run_bass_kernel_spmd

    H = np.asarray(H).astype(np.float32)
    W = np.asarray(W).astype(np.float32)
    bias = np.asarray(bias).astype(np.float32)

    chunks, plan, TOTB, core_arrays = _preprocess(rows, cols, vals)
    nc = _build_program(plan, TOTB)

    # padded node table [NPAD, 128] in (c*128+p)*49+j order
    tbl = np.zeros((NPAD, PADC), np.float32)
    tbl[_row_of_node(np.arange(N_NODES)), :D] = H

    iota128 = np.broadcast_to(np.arange(128, dtype=np.float32), (128, 128)).copy()
    ident = np.eye(128, dtype=np.float32)
    biasb = np.broadcast_to(bias, (128, D)).copy()

    in_maps = []
    for c in range(C):
        widx, dloc_col, vals_col = core_arrays[c]
        # hsh: padded [128, 49*128] partition-major layout of this core's shard
        hsh = np.zeros((128, PAIRS, PADC), np.float32)
        hrows = H[c * SH:(c + 1) * SH]
        for j in range(PAIRS):
            r0, r1 = j * 128, min((j + 1) * 128, SH)
            hsh[0:r1 - r0, j, :D] = hrows[r0:r1]
        in_maps.append({
            "tbl0": tbl,
            "hsh": hsh.reshape(128, PAIRS * PADC),
            "widx": widx,
            "dloc": dloc_col,
            "wval": vals_col,
            "iota128": iota128,
            "ident": ident,
            "wmat": W,
            "biasb": biasb,
        })

    res = run_bass_kernel_spmd(
        nc, in_maps, core_ids=list(range(C)),
        trace=bool(int(os.environ.get("CHEB_TRACE", "0"))),
    )
    last_results = res
    return np.concatenate([res.results[c]["out"] for c in range(C)], axis=0)


# revision 31
# speedup vs baseline: 1.2401x; 1.2401x over previous
"""Chebyshev graph convolution (K=3) on 8 Trainium2 NeuronCores.

Strategy (1D destination partitioning):
- Nodes (destination rows) sharded across 8 cores: core c owns rows
  [c*6250, (c+1)*6250).  Edges partitioned by destination so segment_sum is
  local; per SpMM step the updated node features are AllGather'ed so each
  core can gather arbitrary source rows.
- SpMM on-chip: per 128-destination "pair" (2 halves of 64), edge messages
  are fetched with dma_gather (one gathered 512B row per edge, batches of
  128 edges land one-per-partition), a one-hot selection matrix
  sel[e, d] = vals[e] * (dloc[e] == d) is built on DVE, and the TensorEngine
  computes psum[d, :] += sel.T @ gathered — i.e. the scatter-add.
- Node features live in a partition-major padded layout: node
  (core c, pair j, lane p) -> table row (c*128+p)*49 + j, 128 f32 per row
  (512B, dma_gather elem granularity).  This makes the per-step shard
  writeback a single contiguous [128, 6272] DMA instead of a 6272-descriptor
  strided write.
- dma_gather uses int16 indices (<= 1024 per call), so the 50176-row table
  is addressed in two halves; edges are grouped by (dest pair, dest
  half-of-pair, source half) and padded to 128-edge batches with val=0
  edges.  Batch counts are maxed across cores so all 8 cores run one SPMD
  program.
"""

import sys

if "/opt/trn_rl_repo" not in sys.path:
    sys.path.insert(0, "/opt/trn_rl_repo")

import numpy as np

N_NODES = 50000
D = 96
C = 8  # cores
SH = N_NODES // C  # 6250 rows per core
PAIRS = 49  # ceil(6250/128)
PADSH = PAIRS * 128  # 6272
NPAD = C * PADSH  # 50176 padded table rows
HALF = NPAD // 2  # 25088
PADC = 128  # padded feature columns (512B rows for dma_gather)
CH = 2  # pairs per chunk (gather/sel granularity)

last_results = None  # BassKernelResults of the most recent run (for profiling)


def _row_of_node(g):
    """node id -> padded table row: (c*128 + p)*49 + j for g = c*6250 + j*128 + p."""
    c, r = g // SH, g % SH
    j, p = r // 128, r % 128
    return (c * 128 + p) * PAIRS + j


def _plan_chunks(NB):
    """Global batch layout.  NB[j, q] = batches for (pair j, src half q).
    Batch order: chunk-major, then q, then pair."""
    chunks = [list(range(i, min(i + CH, PAIRS))) for i in range(0, PAIRS, CH)]
    plan = []
    B = 0
    for pj in chunks:
        qspans = {}
        groups = []
        for q in (0, 1):
            q0 = B
            for j in pj:
                nb = int(NB[j, q])
                groups.append((q, j, B, nb))
                B += nb
            qspans[q] = (q0, B)
        plan.append(dict(pairs=pj, groups=groups, qspans=qspans, b0=qspans[0][0], b1=B))
    return chunks, plan, B


def _preprocess(rows, cols, vals):
    """Sort/partition edges, build per-core padded batch arrays."""
    rows = np.asarray(rows).astype(np.int64)
    cols = np.asarray(cols).astype(np.int64)
    vals = np.asarray(vals).astype(np.float32)

    order = np.argsort(rows, kind="stable")
    r_s, c_s, v_s = rows[order], cols[order], vals[order]
    core_bounds = np.searchsorted(r_s, np.arange(C + 1) * SH)

    per_core = []
    counts = np.zeros((C, PAIRS, 2), np.int64)
    for c in range(C):
        s, e = core_bounds[c], core_bounds[c + 1]
        ld = (r_s[s:e] - c * SH).astype(np.int64)
        j = ld // 128
        d128 = (ld % 128).astype(np.float32)
        prow = _row_of_node(c_s[s:e])
        q = (prow >= HALF).astype(np.int64)
        lidx = (prow - q * HALF).astype(np.int64)
        np.add.at(counts[c], (j, q), 1)
        per_core.append((j, q, d128, lidx, v_s[s:e]))

    NB = -(-counts.max(axis=0) // 128)  # ceil
    NB[:, 0] = np.maximum(NB[:, 0], 1)  # every pair has >=1 batch
    chunks, plan, TOTB = _plan_chunks(NB)

    # batch offset of each (j, q) group
    B0 = np.zeros((PAIRS, 2), np.int64)
    for ch in plan:
        for (q, j, b0, nb) in ch["groups"]:
            B0[j, q] = b0

    core_arrays = []
    for c in range(C):
        j, q, d128, lidx, v = per_core[c]
        g_b0 = B0[j, q]  # per-edge group batch offset
        o = np.argsort(g_b0, kind="stable")
        g_sorted = g_b0[o]
        uniq, starts, cnts = np.unique(g_sorted, return_index=True, return_counts=True)
        pos = np.arange(g_sorted.size) - np.repeat(starts, cnts)
        slot = g_sorted * 128 + pos  # global edge slot

        lidx_flat = np.zeros(TOTB * 128, np.int16)
        dloc_col = np.zeros((128, TOTB), np.float32)
        vals_col = np.zeros((128, TOTB), np.float32)
        lane = slot % 128
        bb = slot // 128
        lidx_flat[slot] = lidx[o].astype(np.int16)
        dloc_col[lane, bb] = d128[o]
        vals_col[lane, bb] = v[o]

        # wrapped int16 index tensor: per (chunk, q) span, idx i -> [i%16, i//16]
        widx = np.zeros((16, TOTB * 8), np.int16)
        for ch in plan:
            for qq in (0, 1):
                b0, b1 = ch["qspans"][qq]
                if b1 == b0:
                    continue
                seg = lidx_flat[b0 * 128:b1 * 128]
                n = seg.size
                widx[np.arange(n) % 16, b0 * 8 + np.arange(n) // 16] = seg
        widx = np.tile(widx, (8, 1))
        core_arrays.append((widx, dloc_col, vals_col))

    return chunks, plan, TOTB, core_arrays


def _build_program(plan, TOTB):
    import os
    from concourse import bass, bacc, mybir
    import concourse.tile as tile

    no_cc = bool(int(os.environ.get("CHEB_NO_CC", "0")))
    n_steps = int(os.environ.get("CHEB_STEPS", "3"))
    no_final = bool(int(os.environ.get("CHEB_NO_FINAL", "0")))
    maxch = int(os.environ.get("CHEB_MAXCH", "9999"))
    nqueues = int(os.environ.get("CHEB_QUEUES", "4"))

    f32 = mybir.dt.float32
    nc = bacc.Bacc("TRN2", target_bir_lowering=False, num_devices=C,
                   num_swdge_queues=nqueues)
    gq = [0]  # round-robin gather queue counter

    tbl0 = nc.dram_tensor("tbl0", [NPAD, PADC], f32, kind="ExternalInput")
    hsh_d = nc.dram_tensor("hsh", [128, PAIRS * PADC], f32, kind="ExternalInput")
    widx_d = nc.dram_tensor("widx", [128, TOTB * 8], mybir.dt.int16, kind="ExternalInput")
    dloc_d = nc.dram_tensor("dloc", [128, TOTB], f32, kind="ExternalInput")
    wval_d = nc.dram_tensor("wval", [128, TOTB], f32, kind="ExternalInput")
    iota_d = nc.dram_tensor("iota128", [128, 128], f32, kind="ExternalInput")
    ident_d = nc.dram_tensor("ident", [128, 128], f32, kind="ExternalInput")
    wmat_d = nc.dram_tensor("wmat", [D, D], f32, kind="ExternalInput")
    bias_d = nc.dram_tensor("biasb", [128, D], f32, kind="ExternalInput")
    out_d = nc.dram_tensor("out", [SH, D], f32, kind="ExternalOutput")

    tsh = [nc.dram_tensor(f"tsh{k}", [128, PAIRS * PADC], f32, kind="Internal")
           for k in (1, 2)]
    tfull = [nc.dram_tensor(f"tfull{k}", [NPAD, PADC], f32, kind="Internal",
                            addr_space="Shared") for k in (1, 2)]
    rg = [list(range(C))]

    with tile.TileContext(nc) as tc:
        with (
            tc.tile_pool(name="persist", bufs=1) as pp,
            tc.tile_pool(name="xgp", bufs=2) as xgp,
            tc.tile_pool(name="selp", bufs=2) as selp,
            tc.tile_pool(name="psum", bufs=4, space="PSUM") as psp,
            tc.tile_pool(name="psum2", bufs=2, space="PSUM") as psp2,
        ):
            widx_t = pp.tile([128, TOTB * 8], mybir.dt.int16)
            nc.sync.dma_start(out=widx_t[:], in_=widx_d[:, :])
            dloc_t = pp.tile([128, TOTB], f32)
            nc.sync.dma_start(out=dloc_t[:], in_=dloc_d[:, :])
            wval_t = pp.tile([128, TOTB], f32)
            nc.sync.dma_start(out=wval_t[:], in_=wval_d[:, :])
            iota_t = pp.tile([128, 128], f32)
            nc.sync.dma_start(out=iota_t[:], in_=iota_d[:, :])
            ident_t = pp.tile([128, 128], f32)
            nc.sync.dma_start(out=ident_t[:], in_=ident_d[:, :])
            wmat_t = pp.tile([D, D], f32)
            nc.sync.dma_start(out=wmat_t[:], in_=wmat_d[:, :])
            bias_t = pp.tile([128, D], f32)
            nc.sync.dma_start(out=bias_t[:], in_=bias_d[:, :])

            # T tiles are padded [128, 49*128] so the shard writeback is one
            # contiguous DMA; compute views touch only cols 0:96 of each pair.
            Tp = pp.tile([128, PAIRS * PADC], f32, tag="Tp")
            Tc = pp.tile([128, PAIRS * PADC], f32, tag="Tc")
            U = pp.tile([128, PAIRS * D], f32, tag="U")
            S = pp.tile([128, PAIRS * D], f32, tag="S")

            def v3(t):  # [128, PAIRS, 96] view of a padded tile
                return t[:].rearrange("p (j f) -> p j f", j=PAIRS)[:, :, 0:D]

            nc.gpsimd.memset(Tc[:], 0.0)  # pad cols stay 0 forever
            nc.sync.dma_start(out=Tp[:], in_=hsh_d[:, :])  # T0 = H (padded layout)
            TpV, TcV = v3(Tp), v3(Tc)
            U3 = U[:].rearrange("p (j f) -> p j f", j=PAIRS)
            S3 = S[:].rearrange("p (j f) -> p j f", j=PAIRS)
            nc.vector.tensor_copy(out=U3, in_=TpV)

            def spmm(table):
                """S <- spmm over this core's edges, gathering rows of `table`."""
                for ch in plan[:maxch]:
                    b0c, b1c = ch["b0"], ch["b1"]
                    nbc = b1c - b0c
                    xg = xgp.tile([128, nbc * 128], f32, tag="xg")
                    xg3 = xg[:].rearrange("p (b f) -> p b f", b=nbc)
                    for q in (0, 1):
                        s0, s1 = ch["qspans"][q]
                        # dma_gather accepts at most 1024 indices per call
                        for g0 in range(s0, s1, 8):
                            g1 = min(g0 + 8, s1)
                            nc.gpsimd.dma_gather(
                                out_ap=xg3[:, g0 - b0c:g1 - b0c, :],
                                in_ap=table[q * HALF:(q + 1) * HALF, :],
                                idxs_ap=widx_t[:, g0 * 8:g1 * 8],
                                num_idxs=(g1 - g0) * 128,
                                num_idxs_reg=(g1 - g0) * 128,
                                elem_size=PADC,
                                queue_num=gq[0] % nqueues,
                            )
                            gq[0] += 1
                    sel = selp.tile([128, nbc * 128], f32, tag="sel")
                    sel3 = sel[:].rearrange("p (b f) -> p b f", b=nbc)
                    # build in two halves so matmuls on the first half overlap
                    # the DVE build of the second half
                    for h0 in range(0, nbc, (nbc + 1) // 2):
                        h1 = min(h0 + (nbc + 1) // 2, nbc)
                        nh = h1 - h0
                        iota_b = bass.AP(
                            iota_t[:].tensor, iota_t[:].offset,
                            [iota_t[:].ap[0], [0, nh], [1, 128]],
                        )
                        nc.vector.tensor_tensor(
                            out=sel3[:, h0:h1, :],
                            in0=dloc_t[:, b0c + h0:b0c + h1].to_broadcast(
                                [128, nh, 128]),
                            in1=iota_b,
                            op=mybir.AluOpType.is_equal,
                        )
                        nc.vector.tensor_tensor(
                            out=sel3[:, h0:h1, :],
                            in0=sel3[:, h0:h1, :],
                            in1=wval_t[:, b0c + h0:b0c + h1].to_broadcast(
                                [128, nh, 128]),
                            op=mybir.AluOpType.mult,
                        )
                    for j in ch["pairs"]:
                        ps = psp.tile([128, D], f32, tag="ps")
                        bl = []
                        for (q, jj, gb0, gnb) in ch["groups"]:
                            if jj == j:
                                bl.extend(range(gb0 - b0c, gb0 - b0c + gnb))
                        for i, b in enumerate(bl):
                            nc.tensor.matmul(
                                out=ps[:, :],
                                lhsT=sel[:, b * 128:(b + 1) * 128],
                                rhs=xg[:, b * 128:b * 128 + D],
                                start=(i == 0),
                                stop=(i == len(bl) - 1),
                            )
                        nc.scalar.copy(out=S[:, j * D:(j + 1) * D], in_=ps[:])

            def writeback(k):
                """Tc -> tsh[k] (one contiguous DMA) -> AllGather -> tfull[k]."""
                nc.sync.dma_start(out=tsh[k][:, :], in_=Tc[:])
                nc.gpsimd.collective_compute(
                    "AllGather",
                    mybir.AluOpType.bypass,
                    ins=[tsh[k][:, :]],
                    outs=[tfull[k][:, :]],
                    replica_groups=rg,
                )

            MUL, SUB, ADD = (mybir.AluOpType.mult, mybir.AluOpType.subtract,
                             mybir.AluOpType.add)

            # ---- k=1 : T1 = 2*spmm(H) - T0
            spmm(tbl0)
            nc.vector.scalar_tensor_tensor(
                out=TcV, in0=S3, scalar=2.0, in1=TpV, op0=MUL, op1=SUB)
            nc.vector.tensor_tensor(out=U3, in0=U3, in1=TcV, op=ADD)

            if n_steps >= 2:
                # ---- k=2 : T2 = 2*(2*spmm(T1) - T1) - T0
                if not no_cc:
                    writeback(0)
                spmm(tbl0 if no_cc else tfull[0])
                nc.vector.scalar_tensor_tensor(
                    out=S3, in0=S3, scalar=2.0, in1=TcV, op0=MUL, op1=SUB)
                nc.vector.scalar_tensor_tensor(
                    out=TpV, in0=S3, scalar=2.0, in1=TpV, op0=MUL, op1=SUB)
                Tp, Tc = Tc, Tp
                TpV, TcV = TcV, TpV
                nc.vector.tensor_tensor(out=U3, in0=U3, in1=TcV, op=ADD)

            if n_steps >= 3:
                # ---- k=3 : T3 = 2*(2*spmm(T2) - T2) - T1
                if not no_cc:
                    writeback(1)
                spmm(tbl0 if no_cc else tfull[1])
                nc.vector.scalar_tensor_tensor(
                    out=S3, in0=S3, scalar=2.0, in1=TcV, op0=MUL, op1=SUB)
                nc.vector.scalar_tensor_tensor(
                    out=TpV, in0=S3, scalar=2.0, in1=TpV, op0=MUL, op1=SUB)
                nc.vector.tensor_tensor(out=U3, in0=U3, in1=TpV, op=ADD)

            # ---- out = U @ W + bias, written back per pair
            O = S  # S is dead, reuse as output staging
            for j in range(PAIRS) if not no_final else []:
                pt = psp2.tile([128, 128], f32, tag="pt")
                nc.tensor.transpose(
                    out=pt[0:D, :], in_=U[:, j * D:(j + 1) * D], identity=ident_t[:])
                ut = selp.tile([128, 128], f32, tag="ut")
                nc.scalar.copy(out=ut[0:D, :], in_=pt[0:D, :])
                po = psp2.tile([128, D], f32, tag="po")
                nc.tensor.matmul(
                    out=po[:], lhsT=ut[0:D, :], rhs=wmat_t[:, :],
                    start=True, stop=True)
                nc.vector.tensor_tensor(
                    out=O[:, j * D:(j + 1) * D], in0=po[:], in1=bias_t[:], op=ADD)
                r1 = min((j + 1) * 128, SH)
                eng = nc.sync if j % 2 == 0 else nc.scalar
                eng.dma_start(
                    out=out_d[j * 128:r1, :],
                    in_=O[0:r1 - j * 128, j * D:(j + 1) * D],
                )

    nc.compile()
    return nc


def kernel(rows, cols, vals, H, W, bias):
    global last_results
    import os
    from concourse.bass_utils import run_bass_kernel_spmd

    H = np.asarray(H).astype(np.float32)
    W = np.asarray(W).astype(np.float32)
    bias = np.asarray(bias).astype(np.float32)

    chunks, plan, TOTB, core_arrays = _preprocess(rows, cols, vals)
    nc = _build_program(plan, TOTB)

    # padded node table [NPAD, 128] in (c*128+p)*49+j order
    tbl = np.zeros((NPAD, PADC), np.float32)
    tbl[_row_of_node(np.arange(N_NODES)), :D] = H

    iota128 = np.broadcast_to(np.arange(128, dtype=np.float32), (128, 128)).copy()
    ident = np.eye(128, dtype=np.float32)
    biasb = np.broadcast_to(bias, (128, D)).copy()

    in_maps = []
    for c in range(C):
        widx, dloc_col, vals_col = core_arrays[c]
        # hsh: padded [128, 49*128] partition-major layout of this core's shard
        hsh = np.zeros((128, PAIRS, PADC), np.float32)
        hrows = H[c * SH:(c + 1) * SH]
        for j in range(PAIRS):
            r0, r1 = j * 128, min((j + 1) * 128, SH)
            hsh[0:r1 - r0, j, :D] = hrows[r0:r1]
        in_maps.append({
            "tbl0": tbl,
            "hsh": hsh.reshape(128, PAIRS * PADC),
            "widx": widx,
            "dloc": dloc_col,
            "wval": vals_col,
            "iota128": iota128,
            "ident": ident,
            "wmat": W,
            "biasb": biasb,
        })

    res = run_bass_kernel_spmd(
        nc, in_maps, core_ids=list(range(C)),
        trace=bool(int(os.environ.get("CHEB_TRACE", "0"))),
    )
    last_results = res
    return np.concatenate([res.results[c]["out"] for c in range(C)], axis=0)


# revision 32
# speedup vs baseline: 1.3475x; 1.0866x over previous
"""Chebyshev graph convolution (K=3) on 8 Trainium2 NeuronCores.

Strategy (1D destination partitioning):
- Nodes (destination rows) sharded across 8 cores: core c owns rows
  [c*6250, (c+1)*6250).  Edges partitioned by destination so segment_sum is
  local; per SpMM step the updated node features are AllGather'ed so each
  core can gather arbitrary source rows.
- SpMM on-chip: per 128-destination "pair" (2 halves of 64), edge messages
  are fetched with dma_gather (one gathered 512B row per edge, batches of
  128 edges land one-per-partition), a one-hot selection matrix
  sel[e, d] = vals[e] * (dloc[e] == d) is built on DVE, and the TensorEngine
  computes psum[d, :] += sel.T @ gathered — i.e. the scatter-add.
- Node features live in a partition-major padded layout: node
  (core c, pair j, lane p) -> table row (c*128+p)*49 + j, 128 f32 per row
  (512B, dma_gather elem granularity).  This makes the per-step shard
  writeback a single contiguous [128, 6272] DMA instead of a 6272-descriptor
  strided write.
- dma_gather uses int16 indices (<= 1024 per call), so the 50176-row table
  is addressed in two halves; edges are grouped by (dest pair, dest
  half-of-pair, source half) and padded to 128-edge batches with val=0
  edges.  Batch counts are maxed across cores so all 8 cores run one SPMD
  program.
"""

import sys

if "/opt/trn_rl_repo" not in sys.path:
    sys.path.insert(0, "/opt/trn_rl_repo")

import numpy as np

N_NODES = 50000
D = 96
C = 8  # cores
SH = N_NODES // C  # 6250 rows per core
PAIRS = 49  # ceil(6250/128)
PADSH = PAIRS * 128  # 6272
NPAD = C * PADSH  # 50176 padded table rows
HALF = NPAD // 2  # 25088
PADC = 128  # padded feature columns (512B rows for dma_gather)
CH = 2  # pairs per chunk (gather/sel granularity)

last_results = None  # BassKernelResults of the most recent run (for profiling)


def _row_of_node(g):
    """node id -> padded table row: (c*128 + p)*49 + j for g = c*6250 + j*128 + p."""
    c, r = g // SH, g % SH
    j, p = r // 128, r % 128
    return (c * 128 + p) * PAIRS + j


def _plan_chunks(NB):
    """Global batch layout.  NB[j, q] = batches for (pair j, src half q).
    Batch order: chunk-major, then q, then pair."""
    chunks = [list(range(i, min(i + CH, PAIRS))) for i in range(0, PAIRS, CH)]
    plan = []
    B = 0
    for pj in chunks:
        qspans = {}
        groups = []
        for q in (0, 1):
            q0 = B
            for j in pj:
                nb = int(NB[j, q])
                groups.append((q, j, B, nb))
                B += nb
            qspans[q] = (q0, B)
        plan.append(dict(pairs=pj, groups=groups, qspans=qspans, b0=qspans[0][0], b1=B))
    return chunks, plan, B


def _preprocess(rows, cols, vals):
    """Sort/partition edges, build per-core padded batch arrays."""
    rows = np.asarray(rows).astype(np.int64)
    cols = np.asarray(cols).astype(np.int64)
    vals = np.asarray(vals).astype(np.float32)

    order = np.argsort(rows, kind="stable")
    r_s, c_s, v_s = rows[order], cols[order], vals[order]
    core_bounds = np.searchsorted(r_s, np.arange(C + 1) * SH)

    per_core = []
    counts = np.zeros((C, PAIRS, 2), np.int64)
    for c in range(C):
        s, e = core_bounds[c], core_bounds[c + 1]
        ld = (r_s[s:e] - c * SH).astype(np.int64)
        j = ld // 128
        d128 = (ld % 128).astype(np.float32)
        prow = _row_of_node(c_s[s:e])
        q = (prow >= HALF).astype(np.int64)
        lidx = (prow - q * HALF).astype(np.int64)
        np.add.at(counts[c], (j, q), 1)
        per_core.append((j, q, d128, lidx, v_s[s:e]))

    NB = -(-counts.max(axis=0) // 128)  # ceil
    NB[:, 0] = np.maximum(NB[:, 0], 1)  # every pair has >=1 batch
    chunks, plan, TOTB = _plan_chunks(NB)

    # batch offset of each (j, q) group
    B0 = np.zeros((PAIRS, 2), np.int64)
    for ch in plan:
        for (q, j, b0, nb) in ch["groups"]:
            B0[j, q] = b0

    core_arrays = []
    for c in range(C):
        j, q, d128, lidx, v = per_core[c]
        g_b0 = B0[j, q]  # per-edge group batch offset
        o = np.argsort(g_b0, kind="stable")
        g_sorted = g_b0[o]
        uniq, starts, cnts = np.unique(g_sorted, return_index=True, return_counts=True)
        pos = np.arange(g_sorted.size) - np.repeat(starts, cnts)
        slot = g_sorted * 128 + pos  # global edge slot

        lidx_flat = np.zeros(TOTB * 128, np.int16)
        dloc_col = np.zeros((128, TOTB), np.float32)
        vals_col = np.zeros((128, TOTB), np.float32)
        lane = slot % 128
        bb = slot // 128
        lidx_flat[slot] = lidx[o].astype(np.int16)
        dloc_col[lane, bb] = d128[o]
        vals_col[lane, bb] = v[o]

        # wrapped int16 index tensor: per (chunk, q) span, idx i -> [i%16, i//16]
        widx = np.zeros((16, TOTB * 8), np.int16)
        for ch in plan:
            for qq in (0, 1):
                b0, b1 = ch["qspans"][qq]
                if b1 == b0:
                    continue
                seg = lidx_flat[b0 * 128:b1 * 128]
                n = seg.size
                widx[np.arange(n) % 16, b0 * 8 + np.arange(n) // 16] = seg
        widx = np.tile(widx, (8, 1))
        core_arrays.append((widx, dloc_col, vals_col))

    return chunks, plan, TOTB, core_arrays


def _build_program(plan, TOTB):
    import os
    from concourse import bass, bacc, mybir
    import concourse.tile as tile

    no_cc = bool(int(os.environ.get("CHEB_NO_CC", "0")))
    n_steps = int(os.environ.get("CHEB_STEPS", "3"))
    no_final = bool(int(os.environ.get("CHEB_NO_FINAL", "0")))
    maxch = int(os.environ.get("CHEB_MAXCH", "9999"))
    nqueues = int(os.environ.get("CHEB_QUEUES", "4"))

    f32 = mybir.dt.float32
    nc = bacc.Bacc("TRN2", target_bir_lowering=False, num_devices=C,
                   num_swdge_queues=nqueues)
    gq = [0]  # round-robin gather queue counter

    tbl0 = nc.dram_tensor("tbl0", [NPAD, PADC], f32, kind="ExternalInput")
    hsh_d = nc.dram_tensor("hsh", [128, PAIRS * PADC], f32, kind="ExternalInput")
    widx_d = nc.dram_tensor("widx", [128, TOTB * 8], mybir.dt.int16, kind="ExternalInput")
    dloc_d = nc.dram_tensor("dloc", [128, TOTB], f32, kind="ExternalInput")
    wval_d = nc.dram_tensor("wval", [128, TOTB], f32, kind="ExternalInput")
    iota_d = nc.dram_tensor("iota128", [128, 128], f32, kind="ExternalInput")
    ident_d = nc.dram_tensor("ident", [128, 128], f32, kind="ExternalInput")
    wmat_d = nc.dram_tensor("wmat", [D, D], f32, kind="ExternalInput")
    bias_d = nc.dram_tensor("biasb", [128, D], f32, kind="ExternalInput")
    out_d = nc.dram_tensor("out", [SH, D], f32, kind="ExternalOutput")

    tsh = [nc.dram_tensor(f"tsh{k}", [128, PAIRS * PADC], f32, kind="Internal")
           for k in (1, 2)]
    tfull = [nc.dram_tensor(f"tfull{k}", [NPAD, PADC], f32, kind="Internal",
                            addr_space="Shared") for k in (1, 2)]
    rg = [list(range(C))]

    with tile.TileContext(nc) as tc:
        with (
            tc.tile_pool(name="persist", bufs=1) as pp,
            tc.tile_pool(name="xgp", bufs=2) as xgp,
            tc.tile_pool(name="selp", bufs=2) as selp,
            tc.tile_pool(name="psum", bufs=4, space="PSUM") as psp,
            tc.tile_pool(name="psum2", bufs=2, space="PSUM") as psp2,
        ):
            widx_t = pp.tile([128, TOTB * 8], mybir.dt.int16)
            nc.sync.dma_start(out=widx_t[:], in_=widx_d[:, :])
            dloc_t = pp.tile([128, TOTB], f32)
            nc.sync.dma_start(out=dloc_t[:], in_=dloc_d[:, :])
            wval_t = pp.tile([128, TOTB], f32)
            nc.sync.dma_start(out=wval_t[:], in_=wval_d[:, :])
            iota_t = pp.tile([128, 128], f32)
            nc.sync.dma_start(out=iota_t[:], in_=iota_d[:, :])
            ident_t = pp.tile([128, 128], f32)
            nc.sync.dma_start(out=ident_t[:], in_=ident_d[:, :])
            wmat_t = pp.tile([D, D], f32)
            nc.sync.dma_start(out=wmat_t[:], in_=wmat_d[:, :])
            bias_t = pp.tile([128, D], f32)
            nc.sync.dma_start(out=bias_t[:], in_=bias_d[:, :])

            # T tiles are padded [128, 49*128] so the shard writeback is one
            # contiguous DMA; compute views touch only cols 0:96 of each pair.
            Tp = pp.tile([128, PAIRS * PADC], f32, tag="Tp")
            Tc = pp.tile([128, PAIRS * PADC], f32, tag="Tc")
            U = pp.tile([128, PAIRS * D], f32, tag="U")
            S = pp.tile([128, PAIRS * D], f32, tag="S")

            def v3(t):  # [128, PAIRS, 96] view of a padded tile
                return t[:].rearrange("p (j f) -> p j f", j=PAIRS)[:, :, 0:D]

            nc.gpsimd.memset(Tc[:], 0.0)  # pad cols stay 0 forever
            nc.sync.dma_start(out=Tp[:], in_=hsh_d[:, :])  # T0 = H (padded layout)
            TpV, TcV = v3(Tp), v3(Tc)
            U3 = U[:].rearrange("p (j f) -> p j f", j=PAIRS)
            S3 = S[:].rearrange("p (j f) -> p j f", j=PAIRS)
            nc.vector.tensor_copy(out=U3, in_=TpV)

            def spmm(table):
                """S <- spmm over this core's edges, gathering rows of `table`."""
                for ch in plan[:maxch]:
                    b0c, b1c = ch["b0"], ch["b1"]
                    nbc = b1c - b0c
                    xg = xgp.tile([128, nbc * 128], f32, tag="xg")
                    xg3 = xg[:].rearrange("p (b f) -> p b f", b=nbc)
                    for q in (0, 1):
                        s0, s1 = ch["qspans"][q]
                        # dma_gather accepts at most 1024 indices per call
                        for g0 in range(s0, s1, 8):
                            g1 = min(g0 + 8, s1)
                            nc.gpsimd.dma_gather(
                                out_ap=xg3[:, g0 - b0c:g1 - b0c, :],
                                in_ap=table[q * HALF:(q + 1) * HALF, :],
                                idxs_ap=widx_t[:, g0 * 8:g1 * 8],
                                num_idxs=(g1 - g0) * 128,
                                num_idxs_reg=(g1 - g0) * 128,
                                elem_size=PADC,
                                queue_num=gq[0] % nqueues,
                            )
                            gq[0] += 1
                    sel = selp.tile([128, nbc * 128], f32, tag="sel")
                    sel3 = sel[:].rearrange("p (b f) -> p b f", b=nbc)
                    # build in quarters so matmuls on earlier batches overlap
                    # the DVE build of later ones
                    for h0 in range(0, nbc, (nbc + 3) // 4):
                        h1 = min(h0 + (nbc + 3) // 4, nbc)
                        nh = h1 - h0
                        iota_b = bass.AP(
                            iota_t[:].tensor, iota_t[:].offset,
                            [iota_t[:].ap[0], [0, nh], [1, 128]],
                        )
                        nc.vector.tensor_tensor(
                            out=sel3[:, h0:h1, :],
                            in0=dloc_t[:, b0c + h0:b0c + h1].to_broadcast(
                                [128, nh, 128]),
                            in1=iota_b,
                            op=mybir.AluOpType.is_equal,
                        )
                        nc.vector.tensor_tensor(
                            out=sel3[:, h0:h1, :],
                            in0=sel3[:, h0:h1, :],
                            in1=wval_t[:, b0c + h0:b0c + h1].to_broadcast(
                                [128, nh, 128]),
                            op=mybir.AluOpType.mult,
                        )
                    for j in ch["pairs"]:
                        ps = psp.tile([128, D], f32, tag="ps")
                        bl = []
                        for (q, jj, gb0, gnb) in ch["groups"]:
                            if jj == j:
                                bl.extend(range(gb0 - b0c, gb0 - b0c + gnb))
                        for i, b in enumerate(bl):
                            nc.tensor.matmul(
                                out=ps[:, :],
                                lhsT=sel[:, b * 128:(b + 1) * 128],
                                rhs=xg[:, b * 128:b * 128 + D],
                                start=(i == 0),
                                stop=(i == len(bl) - 1),
                            )
                        nc.scalar.copy(out=S[:, j * D:(j + 1) * D], in_=ps[:])

            def writeback(k):
                """Tc -> tsh[k] (one contiguous DMA) -> AllGather -> tfull[k]."""
                nc.sync.dma_start(out=tsh[k][:, :], in_=Tc[:])
                nc.gpsimd.collective_compute(
                    "AllGather",
                    mybir.AluOpType.bypass,
                    ins=[tsh[k][:, :]],
                    outs=[tfull[k][:, :]],
                    replica_groups=rg,
                )

            MUL, SUB, ADD = (mybir.AluOpType.mult, mybir.AluOpType.subtract,
                             mybir.AluOpType.add)

            # ---- k=1 : T1 = 2*spmm(H) - T0
            spmm(tbl0)
            nc.vector.scalar_tensor_tensor(
                out=TcV, in0=S3, scalar=2.0, in1=TpV, op0=MUL, op1=SUB)
            nc.vector.tensor_tensor(out=U3, in0=U3, in1=TcV, op=ADD)

            if n_steps >= 2:
                # ---- k=2 : T2 = 2*(2*spmm(T1) - T1) - T0
                if not no_cc:
                    writeback(0)
                spmm(tbl0 if no_cc else tfull[0])
                nc.vector.scalar_tensor_tensor(
                    out=S3, in0=S3, scalar=2.0, in1=TcV, op0=MUL, op1=SUB)
                nc.vector.scalar_tensor_tensor(
                    out=TpV, in0=S3, scalar=2.0, in1=TpV, op0=MUL, op1=SUB)
                Tp, Tc = Tc, Tp
                TpV, TcV = TcV, TpV
                nc.vector.tensor_tensor(out=U3, in0=U3, in1=TcV, op=ADD)

            if n_steps >= 3:
                # ---- k=3 : T3 = 2*(2*spmm(T2) - T2) - T1
                if not no_cc:
                    writeback(1)
                spmm(tbl0 if no_cc else tfull[1])
                nc.vector.scalar_tensor_tensor(
                    out=S3, in0=S3, scalar=2.0, in1=TcV, op0=MUL, op1=SUB)
                nc.vector.scalar_tensor_tensor(
                    out=TpV, in0=S3, scalar=2.0, in1=TpV, op0=MUL, op1=SUB)
                nc.vector.tensor_tensor(out=U3, in0=U3, in1=TpV, op=ADD)

            # ---- out = U @ W + bias, written back per pair
            O = S  # S is dead, reuse as output staging
            for j in range(PAIRS) if not no_final else []:
                pt = psp2.tile([128, 128], f32, tag="pt")
                nc.tensor.transpose(
                    out=pt[0:D, :], in_=U[:, j * D:(j + 1) * D], identity=ident_t[:])
                ut = selp.tile([128, 128], f32, tag="ut")
                nc.scalar.copy(out=ut[0:D, :], in_=pt[0:D, :])
                po = psp2.tile([128, D], f32, tag="po")
                nc.tensor.matmul(
                    out=po[:], lhsT=ut[0:D, :], rhs=wmat_t[:, :],
                    start=True, stop=True)
                nc.vector.tensor_tensor(
                    out=O[:, j * D:(j + 1) * D], in0=po[:], in1=bias_t[:], op=ADD)
                r1 = min((j + 1) * 128, SH)
                eng = nc.sync if j % 2 == 0 else nc.scalar
                eng.dma_start(
                    out=out_d[j * 128:r1, :],
                    in_=O[0:r1 - j * 128, j * D:(j + 1) * D],
                )

    nc.compile()
    return nc


def kernel(rows, cols, vals, H, W, bias):
    global last_results
    import os
    from concourse.bass_utils import run_bass_kernel_spmd

    H = np.asarray(H).astype(np.float32)
    W = np.asarray(W).astype(np.float32)
    bias = np.asarray(bias).astype(np.float32)

    chunks, plan, TOTB, core_arrays = _preprocess(rows, cols, vals)
    nc = _build_program(plan, TOTB)

    # padded node table [NPAD, 128] in (c*128+p)*49+j order
    tbl = np.zeros((NPAD, PADC), np.float32)
    tbl[_row_of_node(np.arange(N_NODES)), :D] = H

    iota128 = np.broadcast_to(np.arange(128, dtype=np.float32), (128, 128)).copy()
    ident = np.eye(128, dtype=np.float32)
    biasb = np.broadcast_to(bias, (128, D)).copy()

    in_maps = []
    for c in range(C):
        widx, dloc_col, vals_col = core_arrays[c]
        # hsh: padded [128, 49*128] partition-major layout of this core's shard
        hsh = np.zeros((128, PAIRS, PADC), np.float32)
        hrows = H[c * SH:(c + 1) * SH]
        for j in range(PAIRS):
            r0, r1 = j * 128, min((j + 1) * 128, SH)
            hsh[0:r1 - r0, j, :D] = hrows[r0:r1]
        in_maps.append({
            "tbl0": tbl,
            "hsh": hsh.reshape(128, PAIRS * PADC),
            "widx": widx,
            "dloc": dloc_col,
            "wval": vals_col,
            "iota128": iota128,
            "ident": ident,
            "wmat": W,
            "biasb": biasb,
        })

    res = run_bass_kernel_spmd(
        nc, in_maps, core_ids=list(range(C)),
        trace=bool(int(os.environ.get("CHEB_TRACE", "0"))),
    )
    last_results = res
    return np.concatenate([res.results[c]["out"] for c in range(C)], axis=0)
